# revision 6
# baseline (speedup 1.0000x reference)
"""Trainium2 Bass kernel for nn_DSAGPredictor (dense transposed-softmax attention).

Math (b=1, C=256, H=W=96, n=9216, Z=16):
  xf = x.reshape(256, n)
  q = Wq@xf ; k = Wk@xf ; v = Wv@xf
  S = k^T q                      [n_k, n_q]
  A = softmax(S, axis=q)         (row-normalized over the q axis)
  Y = v @ A
  rel = embd[isWithin, dist+8]   [16, 256]
  pos = rel @ xf                 [16, n]
  final[z] = Wproj[:, :256] @ (Y + x) + Wproj[:, 256] ⊗ pos[z]

Algebra:
  - S = k^T q = x^T (Wk^T Wq xq) = x^T T with T = Wk^T (Wq xq): the k
    projection disappears; raw x tiles streamed from DRAM are the stationary
    operand of the S matmuls.
  - v2 = WprojC @ Wv folded host-side; base = v2 (P/s) + WprojC xq
    accumulates in one persistent PSUM group per (qc, h).
  - Fixed-shift softmax: P = exp(S - 96), s_k = global rowsum over q via
    three small AllReduces pipelined under the S loop; 1/s folded into v2
    rows (gpsimd tensor_scalar divide) right before the PV matmuls.

Sharding: q axis split across 8 cores (1152 columns each). Fused pipeline:
per k-tile (128 tokens) compute S -> exp -> (bf16 P) with P for segment 0
spilled to DRAM and segments 1-2 kept in SBUF; PV accumulates per segment
after its rowsum AllReduce lands. Output (z-outer + residual add) streams
per q-chunk at the end.
"""
import os

from contextlib import ExitStack

import ml_dtypes
import numpy as np

import concourse.bass as bass
import concourse.bacc as bacc
import concourse.tile as tile
from concourse import mybir, bass_utils

N_CORES = 8
CDIM = 256
N_TOK = 9216
NQ = N_TOK // N_CORES   # 1152
QCH = 384
NCH = NQ // QCH         # 3
NKT = N_TOK // 128      # 72
ZDIM = 16
MAXL = 8
C_SHIFT = 96.0
SEGS = [(0, 36), (36, 60), (60, 72)]   # rowsum AllReduce segments
SPILL_SEGS = {0}                       # P segments spilled to DRAM
RG = 3                                 # k-tiles per PV group / spill read
PV0_START_KT = 58                      # first loop slot for seg0 PV bursts

f32 = mybir.dt.float32
f32r = mybir.dt.float32r
bf16 = mybir.dt.bfloat16

_CACHE = {}


def _build_nc():
    nc = bacc.Bacc("TRN2", target_bir_lowering=False, debug=False,
                   num_devices=N_CORES)

    # ---- I/O ----
    xf_d = nc.dram_tensor("xf", [2, 128, N_TOK], f32r, kind="ExternalInput")
    xq_d = nc.dram_tensor("xq", [2, 128, NQ], f32r, kind="ExternalInput")
    wqT_d = nc.dram_tensor("wqT", [2, 128, CDIM], f32r, kind="ExternalInput")
    wk_d = nc.dram_tensor("wk", [2, 128, CDIM], f32r, kind="ExternalInput")
    wv2T_d = nc.dram_tensor("wv2T", [2, 128, CDIM], f32r, kind="ExternalInput")
    wpT_d = nc.dram_tensor("wpT", [2, 128, CDIM], f32r, kind="ExternalInput")
    relT_d = nc.dram_tensor("relT", [2, 128, ZDIM], f32r, kind="ExternalInput")
    wlast_d = nc.dram_tensor("wlast", [1, CDIM], bf16, kind="ExternalInput")
    out_d = nc.dram_tensor("out", [ZDIM, 2, 128, NQ], f32, kind="ExternalOutput")

    seg_len = [b - a for (a, b) in SEGS]

    with tile.TileContext(nc) as tc, ExitStack() as ctx:
        const = ctx.enter_context(tc.tile_pool(name="const", bufs=1))
        big = ctx.enter_context(tc.tile_pool(name="big", bufs=1))
        xin = ctx.enter_context(tc.tile_pool(name="xin", bufs=3))
        dram = ctx.enter_context(tc.tile_pool(name="dram", bufs=1, space="DRAM"))
        psAcc = ctx.enter_context(tc.tile_pool(name="psAcc", bufs=6,
                                               space="PSUM"))

        # ---- persistent SBUF ----
        wv2_r = const.tile([128, 2, CDIM], f32r)
        wl_b = const.tile([1, CDIM], bf16)
        negc = const.tile([128, 1], f32)

        t_s = big.tile([128, 2, NQ], f32r)          # T = Wk^T Wq xq
        # v2^T token-major, one tile per AR segment (keeps the in-place
        # 1/s folds from serializing against later-segment writes under
        # coarse dependency tracking)
        v2seg = [big.tile([128, seg_len[i], CDIM], bf16, name=f"v2s{i}")
                 for i in range(len(SEGS))]
        pos_s = big.tile([ZDIM, NQ], bf16)
        stats = big.tile([128, NKT], f32)
        stats_tot = big.tile([128, NKT], f32)
        recip = big.tile([128, NKT], f32)

        # ---- DRAM scratch ----
        n_sp = SEGS[0][1] - SEGS[0][0]
        pspill = dram.tile([n_sp, 128, NQ], bf16, name="pspill")
        cc_in = [dram.tile([128, seg_len[i]], f32, name=f"cc_in{i}")
                 for i in range(len(SEGS))]
        cc_out = [dram.tile([128, seg_len[i]], f32, addr_space="Shared",
                            name=f"cc_out{i}")
                  for i in range(len(SEGS))]

        def v2ap(kt, hs=slice(None)):
            s = next(i for i, (a, b) in enumerate(SEGS) if a <= kt < b)
            return v2seg[s][:, kt - SEGS[s][0], hs]

        # ---- persistent constants ----
        nc.sync.dma_start(wv2_r[:], wv2T_d[:, :, :].rearrange("h p c -> p h c"))
        nc.sync.dma_start(wl_b[:], wlast_d[:, :])
        nc.vector.memset(negc[:], -C_SHIFT)

        # =========== phase A: T, pos, acc init ===========
        acc = [[None, None] for _ in range(NCH)]
        with tc.tile_pool(name="pA", bufs=1) as pA, \
             tc.tile_pool(name="psA", bufs=2, space="PSUM") as psA:
            xq_r = pA.tile([128, 2, NQ], f32r)
            wq_r = pA.tile([128, 2, CDIM], f32r)
            wk_r = pA.tile([128, 2, CDIM], f32r)
            wp_r = pA.tile([128, 2, CDIM], f32r)
            rel_r = pA.tile([128, 2, ZDIM], f32r)
            t1_s = pA.tile([128, 2, NQ], f32r)
            nc.sync.dma_start(xq_r[:],
                              xq_d[:, :, :].rearrange("h p c -> p h c"))
            nc.sync.dma_start(wq_r[:],
                              wqT_d[:, :, :].rearrange("h p c -> p h c"))
            nc.sync.dma_start(wk_r[:],
                              wk_d[:, :, :].rearrange("h p c -> p h c"))
            nc.sync.dma_start(wp_r[:],
                              wpT_d[:, :, :].rearrange("h p c -> p h c"))
            nc.sync.dma_start(rel_r[:],
                              relT_d[:, :, :].rearrange("h p c -> p h c"))

            for qc in range(NCH):
                qsl = slice(qc * QCH, (qc + 1) * QCH)
                for h in range(2):     # T1 = Wq @ xq
                    hs = slice(h * 128, (h + 1) * 128)
                    ps = psA.tile([128, QCH], f32, tag="mm")
                    nc.tensor.matmul(ps[:], wq_r[:, 0, hs], xq_r[:, 0, qsl],
                                     start=True, stop=False)
                    nc.tensor.matmul(ps[:], wq_r[:, 1, hs], xq_r[:, 1, qsl],
                                     start=False, stop=True)
                    nc.vector.tensor_copy(t1_s[:, h, qsl], ps[:])
                ps_p = psA.tile([ZDIM, QCH], f32, tag="mm")  # pos = rel @ xq
                nc.tensor.matmul(ps_p[:], rel_r[:, 0, :], xq_r[:, 0, qsl],
                                 start=True, stop=False)
                nc.tensor.matmul(ps_p[:], rel_r[:, 1, :], xq_r[:, 1, qsl],
                                 start=False, stop=True)
                nc.vector.tensor_copy(pos_s[:, qsl], ps_p[:])
                for h in range(2):     # T = Wk^T @ T1
                    hs = slice(h * 128, (h + 1) * 128)
                    ps = psA.tile([128, QCH], f32, tag="mm")
                    nc.tensor.matmul(ps[:], wk_r[:, 0, hs], t1_s[:, 0, qsl],
                                     start=True, stop=False)
                    nc.tensor.matmul(ps[:], wk_r[:, 1, hs], t1_s[:, 1, qsl],
                                     start=False, stop=True)
                    nc.vector.tensor_copy(t_s[:, h, qsl], ps[:])
                for h in range(2):     # open accumulators: base = WprojC @ xq
                    hs = slice(h * 128, (h + 1) * 128)
                    ac = psAcc.tile([128, QCH], f32, tag="acc",
                                    name=f"acc{qc}_{h}")
                    nc.tensor.matmul(ac[:], wp_r[:, 0, hs], xq_r[:, 0, qsl],
                                     start=True, stop=False)
                    nc.tensor.matmul(ac[:], wp_r[:, 1, hs], xq_r[:, 1, qsl],
                                     start=False, stop=False)
                    acc[qc][h] = ac

        # main-loop pools (opened after phase A frees its SBUF arena)
        pwin = ctx.enter_context(
            tc.tile_pool(name="pwin", bufs=seg_len[1] + seg_len[2]))
        pout = ctx.enter_context(tc.tile_pool(name="pout", bufs=3))
        pin = ctx.enter_context(tc.tile_pool(name="pin", bufs=2))
        poscp = ctx.enter_context(tc.tile_pool(name="poscp", bufs=1))
        opool = ctx.enter_context(tc.tile_pool(name="opool", bufs=4))

        seg_of = {}
        for i, (a, b) in enumerate(SEGS):
            for kt in range(a, b):
                seg_of[kt] = i
        seg_end = {b - 1: i for i, (a, b) in enumerate(SEGS)}
        pw_tiles = {}

        def _ar_seg(i):
            lo, hi = SEGS[i]
            nc.gpsimd.dma_start(cc_in[i][:], stats[:, lo:hi])
            nc.gpsimd.collective_compute(
                "AllReduce",
                mybir.AluOpType.add,
                replica_groups=[list(range(N_CORES))],
                ins=[cc_in[i][:].opt()],
                outs=[cc_out[i][:].opt()],
            )
            nc.gpsimd.dma_start(stats_tot[:, lo:hi], cc_out[i][:])

        recip_done = set()

        def _pv_group(kt0, g):
            """PV for k-tiles [kt0, kt0+g): fold 1/s into v2, matmul."""
            si = seg_of[kt0]
            spill = si in SPILL_SEGS
            if si not in recip_done:
                # emitted lazily so the DVE queue only waits on the
                # AllReduce at the first PV use of the segment
                recip_done.add(si)
                lo, hi = SEGS[si]
                nc.vector.reciprocal(recip[:, lo:hi], stats_tot[:, lo:hi])
            stg = None
            if spill:
                stg = pin.tile([128, RG, NQ], bf16, tag="pin",
                               name=f"pin{kt0}")
                nc.sync.dma_start(
                    stg[:, 0:g, :],
                    pspill[kt0 - SEGS[0][0]:kt0 - SEGS[0][0] + g]
                    .rearrange("g p c -> p g c"))
            for j in range(g):
                kt = kt0 + j
                nc.vector.tensor_scalar_mul(v2ap(kt), v2ap(kt),
                                            recip[:, kt:kt + 1])
                last = kt == NKT - 1
                for h in range(2):
                    hs = slice(h * 128, (h + 1) * 128)
                    for qc in range(NCH):
                        qsl = slice(qc * QCH, (qc + 1) * QCH)
                        rhs = stg[:, j, qsl] if spill else pw_tiles[kt][:, qsl]
                        nc.tensor.matmul(acc[qc][h][:], v2ap(kt, hs), rhs,
                                         start=False, stop=last)

        # seg0 PV groups interleaved into the S-loop tail
        lo0, hi0 = SEGS[0]
        groups0 = [(kt0, min(RG, hi0 - kt0)) for kt0 in range(lo0, hi0, RG)]
        n_slots = NKT - PV0_START_KT
        pv0_sched = {}
        for i, grp in enumerate(groups0):
            kt_slot = PV0_START_KT + (i * n_slots) // len(groups0)
            pv0_sched.setdefault(kt_slot, []).append(grp)

        with tc.tile_pool(name="psS", bufs=2, space="PSUM") as psS:
            for kt in range(NKT):
                seg = seg_of[kt]
                if kt % 2 == 0:
                    xt = xin.tile([128, 2, 256], f32r, tag="xt")
                    sl = slice(kt * 128, kt * 128 + 256)
                    nc.sync.dma_start(
                        xt[:], xf_d[:, :, sl].rearrange("h p c -> p h c"))
                tsl = slice((kt % 2) * 128, (kt % 2) * 128 + 128)
                if seg in SPILL_SEGS:
                    pt = pout.tile([128, NQ], bf16, tag="pt")
                else:
                    pt = pwin.tile([128, NQ], bf16, tag="pw")
                    pw_tiles[kt] = pt
                for qc in range(NCH):
                    qsl = slice(qc * QCH, (qc + 1) * QCH)
                    ps = psS.tile([128, QCH], f32, tag="stg")
                    nc.tensor.matmul(ps[:], xt[:, 0, tsl], t_s[:, 0, qsl],
                                     start=True, stop=False)
                    nc.tensor.matmul(ps[:], xt[:, 1, tsl], t_s[:, 1, qsl],
                                     start=False, stop=True)
                    nc.scalar.activation(
                        pt[:, qsl], ps[:],
                        mybir.ActivationFunctionType.Exp,
                        bias=negc[:], scale=1.0)
                psv = psS.tile([128, CDIM], f32, tag="stg")   # v2^T tile
                nc.tensor.matmul(psv[:], xt[:, 0, tsl], wv2_r[:, 0, :],
                                 start=True, stop=False)
                nc.tensor.matmul(psv[:], xt[:, 1, tsl], wv2_r[:, 1, :],
                                 start=False, stop=True)
                nc.vector.tensor_copy(v2ap(kt), psv[:])
                nc.vector.tensor_reduce(stats[:, kt:kt + 1], pt[:],
                                        mybir.AxisListType.X,
                                        mybir.AluOpType.add)
                if seg in SPILL_SEGS:
                    nc.sync.dma_start(pspill[kt - SEGS[seg][0]], pt[:])
                if kt in seg_end:
                    _ar_seg(seg_end[kt])
                for (kt0, g) in pv0_sched.get(kt, []):
                    _pv_group(kt0, g)

        # =========== PV for segments 1..2 ===========
        for i in range(1, len(SEGS)):
            lo, hi = SEGS[i]
            for kt0 in range(lo, hi, RG):
                _pv_group(kt0, min(RG, hi - kt0))

        # =========== tail: z-outer + residual combine + output ===========
        with tc.tile_pool(name="ypool", bufs=4) as ypool, \
             tc.tile_pool(name="psO", bufs=2, space="PSUM") as psO:
            for qc in range(NCH):
                qsl = slice(qc * QCH, (qc + 1) * QCH)
                posc = poscp.tile([1, ZDIM, QCH], bf16, tag="posc")
                nc.sync.dma_start(posc[0:1, :, :], pos_s[:, qsl])
                ys = []
                for oh in range(2):
                    y = ypool.tile([128, QCH], f32, tag="y")
                    nc.vector.tensor_copy(y[:], acc[qc][oh][:])
                    ys.append(y)
                for z in range(ZDIM):
                    for oh in range(2):
                        ps_o = psO.tile([128, QCH], f32, tag="zmm")
                        nc.tensor.matmul(
                            ps_o[:], wl_b[0:1, oh * 128:(oh + 1) * 128],
                            posc[0:1, z, :], start=True, stop=True)
                        ot = opool.tile([128, QCH], f32, tag="ot")
                        nc.vector.tensor_add(ot[:], ps_o[:], ys[oh][:])
                        nc.scalar.dma_start(out_d[z, oh, :, qsl], ot[:])

    nc.compile()
    return nc


def _get_nc():
    if "nc" not in _CACHE:
        _CACHE["nc"] = _build_nc()
    return _CACHE["nc"]


def _prep_in_maps(x, Wq, Wk, Wv, embd, Wproj, dist, isWithin):
    x = np.asarray(x, np.float32)
    Wq = np.asarray(Wq, np.float32)
    Wk = np.asarray(Wk, np.float32)
    Wv = np.asarray(Wv, np.float32)
    embd = np.asarray(embd, np.float32)
    Wproj = np.asarray(Wproj, np.float32)
    dist = np.asarray(dist).astype(np.int64)
    isWithin = np.asarray(isWithin).astype(np.int64)

    xf = np.ascontiguousarray(x.reshape(CDIM, N_TOK))
    WprojC = Wproj[:, :CDIM]
    wlast = np.ascontiguousarray(Wproj[:, CDIM]).reshape(1, CDIM)
    Wv2 = WprojC @ Wv
    rel = embd[isWithin, dist + MAXL]            # [16, 256]

    def split2(a):  # [256, m] -> [2, 128, m]
        return np.ascontiguousarray(a.reshape(2, 128, -1), dtype=np.float32)

    common = {
        "xf": split2(xf),
        "wqT": split2(Wq.T),
        "wk": split2(Wk),
        "wv2T": split2(Wv2.T),
        "wpT": split2(WprojC.T),
        "relT": split2(rel.T),
        "wlast": wlast.astype(ml_dtypes.bfloat16),
    }
    in_maps = []
    for c in range(N_CORES):
        m = dict(common)
        m["xq"] = split2(np.ascontiguousarray(xf[:, c * NQ:(c + 1) * NQ]))
        in_maps.append(m)
    return in_maps


def run(inputs, trace=False, tmpdir=None):
    nc = _get_nc()
    in_maps = _prep_in_maps(**inputs)
    res = bass_utils.run_bass_kernel_spmd(
        nc, in_maps, core_ids=list(range(N_CORES)), trace=trace, tmpdir=tmpdir,
    )
    parts = [res.results[c]["out"].reshape(ZDIM, CDIM, NQ)
             for c in range(N_CORES)]
    full = np.concatenate(parts, axis=2).reshape(ZDIM, CDIM, 96, 96)
    return np.ascontiguousarray(full.astype(np.float32)), res


def kernel(**inputs) -> np.ndarray:
    out, _ = run(inputs, trace=bool(int(os.environ.get("KERNEL_TRACE", "0"))))
    return out


# revision 8
# speedup vs baseline: 1.1242x; 1.1242x over previous
"""Trainium2 Bass kernel for nn_DSAGPredictor — fused S/exp/PV pipeline, v3.

S = k^T q = x^T (Wk^T Wq xq): the k projection disappears; raw x tiles
streamed from DRAM are the stationary operand of the S matmuls (2 LDWEIGHTS
per k-tile for S + v2 combined, h-major). One exp per k-tile (strided over
the [128,3,512] staging) with accum_out rowsums. P (bf16) spilled to DRAM.
Three rowsum AllReduces pipelined under the loop; seg0's PV interleaves into
the loop tail (keeps PE duty high for the HAM clock); segs 1-2 PV run
q-chunk-major post-loop with z-outer + output streamed per q-chunk.
"""
import os

from contextlib import ExitStack

import ml_dtypes
import numpy as np

import concourse.bass as bass
import concourse.bacc as bacc
import concourse.tile as tile
from concourse import mybir, bass_utils

N_CORES = 8
CDIM = 256
N_TOK = 9216
NQ = N_TOK // N_CORES   # 1152
QCH = 384
NCH = NQ // QCH         # 3
NKT = N_TOK // 128      # 72
ZDIM = 16
MAXL = 8
C_SHIFT = 96.0
SEGS = [(0, 24), (24, 48), (48, 72)]
RG = 3
PV0_START_KT = 52       # loop slot where seg0 PV bursts begin

f32 = mybir.dt.float32
f32r = mybir.dt.float32r
bf16 = mybir.dt.bfloat16

_CACHE = {}


def _build_nc():
    nc = bacc.Bacc("TRN2", target_bir_lowering=False, debug=False,
                   num_devices=N_CORES)

    xf_d = nc.dram_tensor("xf", [2, 128, N_TOK], f32r, kind="ExternalInput")
    xq_d = nc.dram_tensor("xq", [2, 128, NQ], f32r, kind="ExternalInput")
    wqT_d = nc.dram_tensor("wqT", [2, 128, CDIM], f32r, kind="ExternalInput")
    wk_d = nc.dram_tensor("wk", [2, 128, CDIM], f32r, kind="ExternalInput")
    wv2T_d = nc.dram_tensor("wv2T", [2, 128, CDIM], f32r, kind="ExternalInput")
    wpT_d = nc.dram_tensor("wpT", [2, 128, CDIM], f32r, kind="ExternalInput")
    relT_d = nc.dram_tensor("relT", [2, 128, ZDIM], f32r, kind="ExternalInput")
    wlast_d = nc.dram_tensor("wlast", [1, CDIM], bf16, kind="ExternalInput")
    out_d = nc.dram_tensor("out", [ZDIM, 2, 128, NQ], f32, kind="ExternalOutput")

    seg_len = [b - a for (a, b) in SEGS]
    n_segs = len(SEGS)

    with tile.TileContext(nc) as tc, ExitStack() as ctx:
        const = ctx.enter_context(tc.tile_pool(name="const", bufs=1))
        big = ctx.enter_context(tc.tile_pool(name="big", bufs=1))
        xin = ctx.enter_context(tc.tile_pool(name="xin", bufs=3))
        pout = ctx.enter_context(tc.tile_pool(name="pout", bufs=3))
        pin = ctx.enter_context(tc.tile_pool(name="pin", bufs=3))
        poscp = ctx.enter_context(tc.tile_pool(name="poscp", bufs=1))
        ypool = ctx.enter_context(tc.tile_pool(name="ypool", bufs=2))
        opool = ctx.enter_context(tc.tile_pool(name="opool", bufs=4))
        dram = ctx.enter_context(tc.tile_pool(name="dram", bufs=1, space="DRAM"))

        wv2_r = const.tile([128, 2, CDIM], f32r)
        wp_r = const.tile([128, 2, CDIM], f32r)
        wl_b = const.tile([1, CDIM], bf16)
        negc = const.tile([128, 1], f32)

        xq_r = big.tile([128, 2, NQ], f32r)
        t_s = big.tile([128, 2, NQ], f32r)
        v2seg = [big.tile([128, seg_len[i], CDIM], bf16, name=f"v2s{i}")
                 for i in range(n_segs)]
        pos_s = big.tile([ZDIM, NQ], bf16)
        stats = big.tile([128, NKT], f32)
        stats_tot = big.tile([128, NKT], f32)
        recip = big.tile([128, NKT], f32)

        pspill = [dram.tile([seg_len[i], 128, NQ], bf16, name=f"pspill{i}")
                  for i in range(n_segs)]
        cc_in = [dram.tile([128, seg_len[i]], f32, name=f"cc_in{i}")
                 for i in range(n_segs)]
        cc_out = [dram.tile([128, seg_len[i]], f32, addr_space="Shared",
                            name=f"cc_out{i}")
                  for i in range(n_segs)]

        seg_of = {}
        for i, (a, b) in enumerate(SEGS):
            for kt in range(a, b):
                seg_of[kt] = i
        seg_end = {b - 1: i for i, (a, b) in enumerate(SEGS)}

        def v2ap(kt, hs=slice(None)):
            s = seg_of[kt]
            return v2seg[s][:, kt - SEGS[s][0], hs]

        nc.sync.dma_start(xq_r[:], xq_d[:, :, :].rearrange("h p c -> p h c"))
        nc.sync.dma_start(wv2_r[:], wv2T_d[:, :, :].rearrange("h p c -> p h c"))
        nc.sync.dma_start(wp_r[:], wpT_d[:, :, :].rearrange("h p c -> p h c"))
        nc.sync.dma_start(wl_b[:], wlast_d[:, :])
        nc.vector.memset(negc[:], -C_SHIFT)

        # =========== phase A: T = Wk^T (Wq xq), pos = rel xq ===========
        with tc.tile_pool(name="pA", bufs=1) as pA, \
             tc.tile_pool(name="psA", bufs=4, space="PSUM") as psA:
            wq_r = pA.tile([128, 2, CDIM], f32r)
            wk_r = pA.tile([128, 2, CDIM], f32r)
            rel_r = pA.tile([128, 2, ZDIM], f32r)
            t1_s = pA.tile([128, 2, NQ], f32r)
            nc.sync.dma_start(wq_r[:],
                              wqT_d[:, :, :].rearrange("h p c -> p h c"))
            nc.sync.dma_start(wk_r[:],
                              wk_d[:, :, :].rearrange("h p c -> p h c"))
            nc.sync.dma_start(rel_r[:],
                              relT_d[:, :, :].rearrange("h p c -> p h c"))
            for qc in range(NCH):
                qsl = slice(qc * QCH, (qc + 1) * QCH)
                for h in range(2):
                    hs = slice(h * 128, (h + 1) * 128)
                    ps = psA.tile([128, QCH], f32, tag="mm")
                    nc.tensor.matmul(ps[:], wq_r[:, 0, hs], xq_r[:, 0, qsl],
                                     start=True, stop=False)
                    nc.tensor.matmul(ps[:], wq_r[:, 1, hs], xq_r[:, 1, qsl],
                                     start=False, stop=True)
                    nc.vector.tensor_copy(t1_s[:, h, qsl], ps[:])
                ps_p = psA.tile([ZDIM, QCH], f32, tag="mm")
                nc.tensor.matmul(ps_p[:], rel_r[:, 0, :], xq_r[:, 0, qsl],
                                 start=True, stop=False)
                nc.tensor.matmul(ps_p[:], rel_r[:, 1, :], xq_r[:, 1, qsl],
                                 start=False, stop=True)
                nc.vector.tensor_copy(pos_s[:, qsl], ps_p[:])
                for h in range(2):
                    hs = slice(h * 128, (h + 1) * 128)
                    ps = psA.tile([128, QCH], f32, tag="mm")
                    nc.tensor.matmul(ps[:], wk_r[:, 0, hs], t1_s[:, 0, qsl],
                                     start=True, stop=False)
                    nc.tensor.matmul(ps[:], wk_r[:, 1, hs], t1_s[:, 1, qsl],
                                     start=False, stop=True)
                    nc.vector.tensor_copy(t_s[:, h, qsl], ps[:])

        def _ar_seg(i):
            lo, hi = SEGS[i]
            nc.gpsimd.dma_start(cc_in[i][:], stats[:, lo:hi])
            nc.gpsimd.collective_compute(
                "AllReduce",
                mybir.AluOpType.add,
                replica_groups=[list(range(N_CORES))],
                ins=[cc_in[i][:].opt()],
                outs=[cc_out[i][:].opt()],
            )
            nc.gpsimd.dma_start(stats_tot[:, lo:hi], cc_out[i][:])

        recip_done = set()
        fold_done = set()

        def _fold(si):
            """Reciprocal + fold 1/s into v2 rows for a whole segment (DVE)."""
            if si in fold_done:
                return
            fold_done.add(si)
            lo, hi = SEGS[si]
            nc.vector.reciprocal(recip[:, lo:hi], stats_tot[:, lo:hi])
            for kt in range(lo, hi):
                nc.vector.tensor_scalar_mul(v2ap(kt), v2ap(kt),
                                            recip[:, kt:kt + 1])

        acc = [[None, None] for _ in range(NCH)]

        # =========== main loop: S -> exp -> spill ===========
        def _open_accs():
            for qc in range(NCH):
                qsl = slice(qc * QCH, (qc + 1) * QCH)
                for h in range(2):
                    hs = slice(h * 128, (h + 1) * 128)
                    ac = psAcc.tile([128, QCH], f32, tag="acc",
                                    name=f"acc{qc}_{h}")
                    nc.tensor.matmul(ac[:], wp_r[:, 0, hs], xq_r[:, 0, qsl],
                                     start=True, stop=False)
                    nc.tensor.matmul(ac[:], wp_r[:, 1, hs], xq_r[:, 1, qsl],
                                     start=False, stop=False)
                    acc[qc][h] = ac

        with tc.tile_pool(name="psS", bufs=2, space="PSUM") as psS, \
             tc.tile_pool(name="psV", bufs=2, space="PSUM") as psV:
            for kt in range(NKT):
                seg = seg_of[kt]
                if kt % 2 == 0:
                    xt = xin.tile([128, 2, 256], f32r, tag="xt")
                    sl = slice(kt * 128, kt * 128 + 256)
                    nc.sync.dma_start(
                        xt[:], xf_d[:, :, sl].rearrange("h p c -> p h c"))
                tsl = slice((kt % 2) * 128, (kt % 2) * 128 + 128)
                ps = psS.tile([128, NCH, 512], f32, tag="stg")
                psv = psV.tile([128, CDIM], f32, tag="v2s")
                for h in range(2):
                    st, sp = h == 0, h == 1
                    for qc in range(NCH):
                        nc.tensor.matmul(ps[:, qc, 0:QCH], xt[:, h, tsl],
                                         t_s[:, h, qc * QCH:(qc + 1) * QCH],
                                         start=st, stop=sp)
                    nc.tensor.matmul(psv[:], xt[:, h, tsl], wv2_r[:, h, :],
                                     start=st, stop=sp)
                pt = pout.tile([128, NQ], bf16, tag="pt")
                pt3 = pt[:].rearrange("p (c q) -> p c q", c=NCH)
                nc.scalar.activation(
                    pt3[:, :, :], ps[:, :, 0:QCH],
                    mybir.ActivationFunctionType.Exp,
                    bias=negc[:], scale=1.0,
                    accum_out=stats[:, kt:kt + 1])
                nc.vector.tensor_copy(v2ap(kt), psv[:])
                nc.sync.dma_start(pspill[seg][kt - SEGS[seg][0]], pt[:])
                if kt in seg_end:
                    _ar_seg(seg_end[kt])

        psAcc = ctx.enter_context(tc.tile_pool(name="psAcc", bufs=6,
                                               space="PSUM"))
        _open_accs()

        def _pv_qc(qc, si, stop_seg):
            """Post-loop PV for one (q-chunk, segment)."""
            _fold(si)
            lo, hi = SEGS[si]
            qsl = slice(qc * QCH, (qc + 1) * QCH)
            for kt0 in range(lo, hi, RG):
                g = min(RG, hi - kt0)
                stg = pin.tile([128, RG, QCH], bf16, tag="pinq",
                               name=f"pinq{qc}_{kt0}")
                nc.sync.dma_start(
                    stg[:, 0:g, :],
                    pspill[si][kt0 - lo:kt0 - lo + g, :, qsl]
                    .rearrange("g p c -> p g c"))
                for j in range(g):
                    kt = kt0 + j
                    last = stop_seg and (kt == hi - 1)
                    for h in range(2):
                        hs = slice(h * 128, (h + 1) * 128)
                        nc.tensor.matmul(acc[qc][h][:], v2ap(kt, hs),
                                         stg[:, j, :],
                                         start=False, stop=last)

        def _tail_qc(qc, psO):
            qsl = slice(qc * QCH, (qc + 1) * QCH)
            posc = poscp.tile([1, ZDIM, QCH], bf16, tag="posc")
            nc.sync.dma_start(posc[0:1, :, :], pos_s[:, qsl])
            ys = []
            for oh in range(2):
                y = ypool.tile([128, QCH], f32, tag="y")
                nc.vector.tensor_copy(y[:], acc[qc][oh][:])
                ys.append(y)
            for z in range(ZDIM):
                for oh in range(2):
                    ps_o = psO.tile([128, QCH], f32, tag="zmm")
                    nc.tensor.matmul(
                        ps_o[:], wl_b[0:1, oh * 128:(oh + 1) * 128],
                        posc[0:1, z, :], start=True, stop=True)
                    ot = opool.tile([128, QCH], f32, tag="ot")
                    nc.vector.tensor_add(ot[:], ps_o[:], ys[oh][:])
                    nc.scalar.dma_start(out_d[z, oh, :, qsl], ot[:])

        # =========== post-loop: seg1 PV, then seg2 + tail per q-chunk ======
        with tc.tile_pool(name="psO", bufs=2, space="PSUM") as psO:
            for qc in range(NCH):
                _pv_qc(qc, 0, stop_seg=False)
                _pv_qc(qc, 1, stop_seg=False)
            for qc in range(NCH):
                _pv_qc(qc, 2, stop_seg=True)
                _tail_qc(qc, psO)

    nc.compile()
    return nc


def _get_nc():
    if "nc" not in _CACHE:
        _CACHE["nc"] = _build_nc()
    return _CACHE["nc"]


def _prep_in_maps(x, Wq, Wk, Wv, embd, Wproj, dist, isWithin):
    x = np.asarray(x, np.float32)
    Wq = np.asarray(Wq, np.float32)
    Wk = np.asarray(Wk, np.float32)
    Wv = np.asarray(Wv, np.float32)
    embd = np.asarray(embd, np.float32)
    Wproj = np.asarray(Wproj, np.float32)

    xf = np.ascontiguousarray(x.reshape(CDIM, N_TOK))
    WprojC = Wproj[:, :CDIM]
    wlast = np.ascontiguousarray(Wproj[:, CDIM]).reshape(1, CDIM)
    Wv2 = WprojC @ Wv
    dist = np.asarray(dist).astype(np.int64)
    isWithin = np.asarray(isWithin).astype(np.int64)
    rel = embd[isWithin, dist + MAXL]

    def split2(a):
        return np.ascontiguousarray(a.reshape(2, 128, -1), dtype=np.float32)

    common = {
        "xf": split2(xf),
        "wqT": split2(Wq.T),
        "wk": split2(Wk),
        "wv2T": split2(Wv2.T),
        "wpT": split2(WprojC.T),
        "relT": split2(rel.T),
        "wlast": wlast.astype(ml_dtypes.bfloat16),
    }
    in_maps = []
    for c in range(N_CORES):
        m = dict(common)
        m["xq"] = split2(np.ascontiguousarray(xf[:, c * NQ:(c + 1) * NQ]))
        in_maps.append(m)
    return in_maps


def run(inputs, trace=False, tmpdir=None):
    nc = _get_nc()
    in_maps = _prep_in_maps(**inputs)
    res = bass_utils.run_bass_kernel_spmd(
        nc, in_maps, core_ids=list(range(N_CORES)), trace=trace, tmpdir=tmpdir,
    )
    parts = [res.results[c]["out"].reshape(ZDIM, CDIM, NQ)
             for c in range(N_CORES)]
    full = np.concatenate(parts, axis=2).reshape(ZDIM, CDIM, 96, 96)
    return np.ascontiguousarray(full.astype(np.float32)), res


def kernel(**inputs) -> np.ndarray:
    out, _ = run(inputs, trace=bool(int(os.environ.get("KERNEL_TRACE", "0"))))
    return out
